# revision 39
# baseline (speedup 1.0000x reference)
"""Causal multi-head attention block (16 heads, dim 1024) on 8 TRN2 NeuronCores.

Sharding: tensor-parallel over heads — core c computes heads {2c, 2c+1}:
  q/k/v projections with the 128-column weight slices, causal attention,
  and a partial output projection with the matching 128 Wout rows.
Host sums the 8 partial outputs and adds the bias.

Per-core dataflow (per batch of 2048 tokens):
  phase12: xT (dim-major) DMA'd directly from DRAM (x is pre-transposed
           on the host); qT/kT/vT = W.T @ xT (feature-major, 2 heads
           packed on 128 partitions); vT -> PE-transpose -> v_aug
           (tok-major, 65 cols/head: 64 v + ones).
  phase34: scores TRANSPOSED: dotsT[j,i] = kT.T @ qT, causal mask added
           via a second matmul in the same accumulation group, exp on ACT
           (no max-subtraction needed; exponents are small) -> attnT;
           AV: outT = v_aug.T @ attnT accumulated over j-tiles; psum row 64
           = softmax denominators (ones-column trick). Normalize, then
           output projection per 128-token tile, DMA out.

Engines run their instruction streams IN ORDER, so phase12(b+1) emission is
interleaved with phase34(b) to fill PE gaps left by exp latency and to keep
DMA/DVE/ACT busy concurrently (software pipelining at emission order).
"""
import numpy as np
import ml_dtypes
from contextlib import ExitStack, nullcontext

import concourse.bacc as bacc
import concourse.mybir as mybir
import concourse.tile as tile
import concourse.bass_utils as bass_utils
from concourse import masks

F32 = mybir.dt.float32
F32R = mybir.dt.float32r
BF16 = mybir.dt.bfloat16
FP16 = mybir.dt.float16

B = 4            # batches
T = 2048         # tokens per batch
DIM = 1024
NT = T // 128    # token tiles per batch (16)
KT = DIM // 128  # contraction tiles (8)
NCHUNK = T // 512  # 512-col i-chunks per batch (4)
SCALE = DIM ** -0.5  # 1/32 — NOTE: full dim, not head dim (matches reference)
MASK_NEG = -1.0e9

DEFAULT_MMDT = "bf16"
_CACHED = {}


def build_kernel(repeat=None, mmdt=None, nbatches=None, interleave=True):
    mmdt = mmdt or DEFAULT_MMDT
    MMDT = {"f32r": F32R, "bf16": BF16}[mmdt]
    NB = nbatches if nbatches is not None else B

    nc = bacc.Bacc("TRN2", target_bir_lowering=False, debug=False, num_devices=8)

    xdt = F32R if MMDT == F32R else BF16
    odt = F32 if MMDT == F32R else FP16
    # x arrives pre-transposed (dim-major) from the host: xT[dim, B*T]
    x_d = nc.dram_tensor("x", [DIM, B * T], xdt, kind="ExternalInput").ap()
    wq_d = nc.dram_tensor("wq", [DIM, 128], xdt, kind="ExternalInput").ap()
    wk_d = nc.dram_tensor("wk", [DIM, 128], xdt, kind="ExternalInput").ap()
    wv_d = nc.dram_tensor("wv", [DIM, 128], xdt, kind="ExternalInput").ap()
    wo_d = nc.dram_tensor("wo", [128, DIM], xdt, kind="ExternalInput").ap()
    out_d = nc.dram_tensor("out", [B * T, DIM], odt, kind="ExternalOutput").ap()

    with tile.TileContext(nc) as tc, ExitStack() as ctx:
        cp = ctx.enter_context(tc.tile_pool(name="const", bufs=1))
        xT_p = ctx.enter_context(tc.tile_pool(name="xT", bufs=2))
        qT_p = ctx.enter_context(tc.tile_pool(name="qT", bufs=2))
        kT_p = ctx.enter_context(tc.tile_pool(name="kT", bufs=2))
        vT_p = ctx.enter_context(tc.tile_pool(name="vT", bufs=2))
        vaug_p = ctx.enter_context(tc.tile_pool(name="vaug", bufs=2))
        attnT_p = ctx.enter_context(tc.tile_pool(name="attnT", bufs=6))
        recip_p = ctx.enter_context(tc.tile_pool(name="recip", bufs=4))
        avu_p = ctx.enter_context(tc.tile_pool(name="avu", bufs=4))
        rbc_p = ctx.enter_context(tc.tile_pool(name="rbc", bufs=2))
        outT_p = ctx.enter_context(tc.tile_pool(name="outT", bufs=2))
        osb_p = ctx.enter_context(tc.tile_pool(name="osb", bufs=3))
        mm_ps = ctx.enter_context(tc.tile_pool(name="mmps", bufs=2, space="PSUM"))
        dots_ps = ctx.enter_context(tc.tile_pool(name="dotsps", bufs=2, space="PSUM"))
        av_ps_p = ctx.enter_context(tc.tile_pool(name="avps", bufs=2, space="PSUM"))

        # ---- constants ----
        ident32 = cp.tile([128, 128], F32, tag="ident32")
        masks.make_identity(nc, ident32[:])
        ident = cp.tile([128, 128], MMDT, tag="ident")
        nc.vector.tensor_copy(ident[:], ident32[:])

        ones32 = cp.tile([128, 2 * NT], F32, tag="ones32")
        nc.gpsimd.memset(ones32[:], 1.0)

        # mask01[j, i] = 1 where j <= i else 0 (causal keep-mask for the
        # diagonal 128x128 blocks; applied by DVE multiply instead of gpsimd
        # affine_select so gpsimd can't convoy the normalization chain)
        mask32 = cp.tile([128, 128], F32, tag="mask32")
        nc.gpsimd.memset(mask32[:], 1.0)
        nc.gpsimd.affine_select(
            out=mask32[:], in_=mask32[:],
            compare_op=mybir.AluOpType.is_ge, fill=0.0,
            base=0, pattern=[[1, 128]], channel_multiplier=-1,
        )
        mask01 = cp.tile([128, 128], MMDT, tag="mask01")
        nc.vector.tensor_copy(mask01[:], mask32[:])

        # ---- weights ----
        wq_sb = cp.tile([128, KT * 128], MMDT, tag="wq")
        wk_sb = cp.tile([128, KT * 128], MMDT, tag="wk")
        wv_sb = cp.tile([128, KT * 128], MMDT, tag="wv")
        wo_sb = cp.tile([128, DIM], MMDT, tag="wo")
        # weights on the sync ring so they land in parallel with batch 0's
        # xT tiles (which go on the scalar ring)
        for w_sb, w_d in ((wq_sb, wq_d), (wk_sb, wk_d), (wv_sb, wv_d)):
            nc.sync.dma_start(w_sb[:].rearrange("p (kt m) -> p kt m", kt=KT),
                              w_d.rearrange("(kt p) m -> p kt m", p=128))
        nc.sync.dma_start(wo_sb[:], wo_d)

        state = {}  # per-batch qT/kT/vaug handles

        def phase12_steps(b):
            """xT DMA + projections + v_aug for batch b. Yields between steps."""
            t0 = b * T
            xT = xT_p.tile([128, KT * T], MMDT, tag="xT", name="xT")
            # all 8 k-tile DMAs up front on the scalar ring (bulk traffic),
            # keeping the sync ring free for small latency-critical DMAs
            for kt in range(KT):
                nc.scalar.dma_start(xT[:, kt * T:(kt + 1) * T],
                                    x_d[kt * 128:(kt + 1) * 128, t0:t0 + T])
            yield
            qkv = []
            for w_sb, pool, tag in ((wq_sb, qT_p, "qT"), (wk_sb, kT_p, "kT"),
                                    (wv_sb, vT_p, "vT")):
                dest = pool.tile([128, T], MMDT, tag=tag, name=tag)
                qkv.append(dest)
                for ch in range(NCHUNK):
                    pp = mm_ps.tile([128, 512], F32, tag="mm", name="pp")
                    for kt in range(KT):
                        nc.tensor.matmul(
                            pp[:], w_sb[:, kt * 128:(kt + 1) * 128],
                            xT[:, kt * T + ch * 512: kt * T + (ch + 1) * 512],
                            start=(kt == 0), stop=(kt == KT - 1))
                    nc.vector.tensor_copy(dest[:, ch * 512:(ch + 1) * 512], pp[:])
                    yield
            qT, kT_t, vT = qkv
            vaug = vaug_p.tile([128, NT * 130], MMDT, tag="vaug", name="vaug")
            nvg = 4 if MMDT == F32R else 8
            for jtg in range(NT // nvg):
                tp = mm_ps.tile([128, 512], F32, tag="mm", name="tpv")
                tpv = tp[:].bitcast(MMDT)
                for j in range(nvg):
                    jt = nvg * jtg + j
                    nc.tensor.transpose(tpv[:, j * 128:(j + 1) * 128],
                                        vT[:, jt * 128:(jt + 1) * 128], ident[:])
                vv = vaug[:].rearrange("p (jt c) -> p jt c", c=130)
                src = tpv[:, 0:nvg * 128].rearrange("p (j c) -> p j c", j=nvg)
                nc.vector.tensor_copy(vv[:, nvg * jtg:nvg * (jtg + 1), 0:64], src[:, :, 0:64])
                nc.vector.tensor_copy(vv[:, nvg * jtg:nvg * (jtg + 1), 65:129], src[:, :, 64:128])
                yield
            nc.vector.tensor_copy(
                vaug[:].rearrange("p (u c) -> p u c", c=65)[:, :, 64:65],
                ones32[:].rearrange("p (u o) -> p u o", o=1))
            state[b] = (qT, kT_t, vaug)

        def phase34_steps(b):
            """Attention + deferred chunk-wise output projection for batch b."""
            t0 = b * T
            qT, kT_t, vaug = state.pop(b)
            outT = outT_p.tile([128, T], MMDT, tag="outT", name="outT")
            pending = []  # token-tiles whose output projection is deferred

            def outproj(tt):
                # emitted during the NEXT chunk's jp loop so the PE stream
                # has independent scores work between stt and these matmuls
                osb = osb_p.tile([128, DIM], odt, tag="osb", name="osb")
                for half in (0, 1):
                    po = mm_ps.tile([128, 512], F32, tag="mm", name="po")
                    nc.tensor.matmul(po[:], outT[:, tt * 128:(tt + 1) * 128],
                                     wo_sb[:, half * 512:(half + 1) * 512],
                                     start=True, stop=True)
                    nc.vector.tensor_copy(osb[:, half * 512:(half + 1) * 512], po[:])
                nc.sync.dma_start(out_d[t0 + tt * 128: t0 + (tt + 1) * 128, :],
                                  osb[:])

            for c in range(NCHUNK):
                njt = 4 * (c + 1)
                avp = {h: av_ps_p.tile([65, 512], F32, tag="av", name=f"avp{h}")
                       for h in (0, 1)}
                for jp in range(njt // 2):
                    jts = (2 * jp, 2 * jp + 1)
                    offs = [max(512 * c, jt * 128) - 512 * c for jt in jts]
                    dps, ats = {}, {}
                    for h in (0, 1):
                        dps[h] = dots_ps.tile([128, 1024], F32, tag="dots", name=f"dp{h}")
                    # j outer, h inner: adjacent matmuls hit disjoint PE row
                    # groups (h0 rows 0-63, h1 rows 64-127) and run concurrently.
                    # Full 512-column writes keep everything the exp reads
                    # initialized (AV reads the trimmed per-j range, so the
                    # masked columns are never consumed).
                    for j, jt in enumerate(jts):
                        for h in (0, 1):
                            nc.tensor.matmul(
                                dps[h][:, j * 512: (j + 1) * 512],
                                kT_t[64 * h:64 * h + 64, jt * 128:(jt + 1) * 128],
                                qT[64 * h:64 * h + 64, 512 * c:512 * (c + 1)],
                                start=True, stop=True)
                    for h in (0, 1):
                        at = attnT_p.tile([128, 1024], MMDT, tag="at", name=f"at{h}")
                        nc.scalar.activation(at[:, offs[0]:1024], dps[h][:, offs[0]:1024],
                                             mybir.ActivationFunctionType.Exp,
                                             bias=0.0, scale=float(SCALE))
                        for j, jt in enumerate(jts):
                            if jt >= 4 * c:  # zero invalid (j > i) entries
                                sl = at[:, j * 512 + offs[j]: j * 512 + offs[j] + 128]
                                nc.vector.tensor_mul(sl, sl, mask01[:])
                        ats[h] = at
                    for h in (0, 1):
                        for j, jt in enumerate(jts):
                            off = offs[j]
                            nc.tensor.matmul(
                                avp[h][:, off:512],
                                vaug[:, jt * 130 + 65 * h: jt * 130 + 65 * h + 65],
                                ats[h][:, j * 512 + off: (j + 1) * 512],
                                start=(jt == 0), stop=(jt == njt - 1))
                    if pending:
                        outproj(pending.pop(0))
                    yield
                for h in (0, 1):
                    # evacuate the AV accumulator to SBUF right away: the
                    # psum bank is freed after these two copies (~1.4us)
                    # instead of after the whole normalize chain (~4.4us),
                    # unblocking the next chunk's AV matmuls early.
                    dn = recip_p.tile([1, 512], F32, tag="dn", name="dn")
                    nc.scalar.copy(dn[:], avp[h][64:65, :])
                    avu = avu_p.tile([64, 512], F32, tag="avu", name="avu")
                    nc.vector.tensor_copy(avu[:], avp[h][0:64, :])
                    rc = recip_p.tile([1, 512], F32, tag="recip", name="rc")
                    nc.vector.reciprocal_approx_fast(rc[:], dn[:])
                    rb = rbc_p.tile([64, 512], F32, tag="rbc", name="rb")
                    nc.gpsimd.partition_broadcast(rb[:], rc[:])
                    nc.vector.scalar_tensor_tensor(
                        outT[64 * h:64 * h + 64, c * 512:(c + 1) * 512],
                        avu[:], 1.0, rb[:],
                        op0=mybir.AluOpType.mult, op1=mybir.AluOpType.mult)
                pending += [4 * c, 4 * c + 1, 4 * c + 2, 4 * c + 3]
                yield
            while pending:
                outproj(pending.pop(0))
                yield

        def drive(pairs):
            """Interleave weighted (gen, n_steps) emission generators.

            Advances whichever generator is furthest behind schedule so all
            finish together -- phase12 filler work (projections for batch
            b+1) is spread across the WHOLE of phase34(b), keeping the PE
            fed during softmax/normalization dependency stalls.
            """
            live = [[g, n, 0] for p in pairs if p is not None for g, n in [p]]
            while live:
                e = min(live, key=lambda e: e[2] / e[1])
                try:
                    next(e[0])
                    e[2] += 1
                except StopIteration:
                    live.remove(e)

        # emission step counts (yields per generator), for pacing
        P12_STEPS = 1 + 3 * NCHUNK + NT // (4 if MMDT == F32R else 8)
        P34_STEPS = sum(2 * (c + 1) + 1 for c in range(NCHUNK)) + 4

        rep_ctx = tc.For_i(0, repeat, 1) if repeat is not None else nullcontext()
        with rep_ctx:
            if interleave:
                for b in range(NB + 1):
                    drive([(phase12_steps(b), P12_STEPS) if b < NB else None,
                           (phase34_steps(b - 1), P34_STEPS) if b >= 1 else None])
            else:
                for b in range(NB):
                    drive([(phase12_steps(b), P12_STEPS)])
                    drive([(phase34_steps(b), P34_STEPS)])

    nc.compile()
    return nc


def kernel(x, Wq, Wkv, Wout, bout):
    """Full inputs -> full output. Shards across 8 NeuronCores internally."""
    if "nc" not in _CACHED:
        _CACHED["nc"] = build_kernel()
    nc = _CACHED["nc"]

    hdt = np.float32 if DEFAULT_MMDT == "f32r" else ml_dtypes.bfloat16
    # pre-transpose x on the host: kernel reads dim-major xT[dim, B*T]
    x = np.asarray(x, dtype=np.float32).reshape(B * T, DIM).T
    x = np.ascontiguousarray(x).astype(hdt)
    Wq = np.asarray(Wq, dtype=np.float32).astype(hdt)
    Wkv = np.asarray(Wkv, dtype=np.float32).astype(hdt)
    Wout = np.asarray(Wout, dtype=np.float32).astype(hdt)
    bout = np.asarray(bout, dtype=np.float32)

    in_maps = []
    for c in range(8):
        s = slice(128 * c, 128 * (c + 1))
        in_maps.append({
            "x": x,
            "wq": np.ascontiguousarray(Wq[:, s]),
            "wk": np.ascontiguousarray(Wkv[:, :DIM][:, s]),
            "wv": np.ascontiguousarray(Wkv[:, DIM:][:, s]),
            "wo": np.ascontiguousarray(Wout[s, :]),
        })

    res = bass_utils.run_bass_kernel_spmd(nc, in_maps, core_ids=list(range(8)))
    acc = res.results[0]["out"].astype(np.float64)
    for c in range(1, 8):
        acc += res.results[c]["out"]
    out = (acc + bout.astype(np.float64)).astype(np.float32)
    return out.reshape(B, T, DIM)

